# revision 16
# baseline (speedup 1.0000x reference)
"""GCN layer (GCNConv + BatchNorm + ReLU) as a distributed Bass kernel on 8 TRN2 NeuronCores.

Strategy:
  - Aggregation commutes with the linear transform: A_norm @ (x @ W.T) == (A_norm @ x) @ W.T,
    and the GCN symmetric norm factorizes: norm_e = dinv[src]*dinv[dst]. So we prescale
    x~ = x * dinv[:,None] on the host (bf16), gather x~[src] rows on device with
    dma_gather, aggregate per dest tile via TensorE matmuls against on-the-fly one-hot
    selection matrices (psum[f, d] += G[e, f].T @ S[e, d], S folds dinv[dst]), then apply
    W.T on device.
  - Nodes (and their incident in-edges + self loop) are sharded across 8 cores into
    (tiles_per_core x 128)-slot dest tiles, load-balanced by in-degree.
  - dma_gather uses int16 indices, so x~ is split into row-ranges of <=25000 rows; each
    dest tile gets a fixed per-range chunk budget so the single SPMD program is uniform
    across cores (pad slots gather row 0 and are masked by the selection matrix).
  - BatchNorm batch stats ([128,2] floats) are all-reduced across the 8 cores, then
    scale/shift + ReLU applied in-place and stored with one DMA.
  - b is accepted but mathematically cancels inside BatchNorm.
"""

import numpy as np
import ml_dtypes

import concourse.bass as bass
import concourse.bacc as bacc
import concourse.mybir as mybir
import concourse.tile as tile
from concourse.bass_utils import run_bass_kernel_spmd

N_NODES = 100000
D = 128
N_CORES = 8
TILES_PER_CORE = 98
CH_PER_G = 8  # chunks (128 idxs each) per dma_gather; 1024 idxs = HW ring limit
TABLE_ROWS = 25000  # rows per gather sub-table (int16 index limit)
BN_EPS = 1e-5

BF16 = mybir.dt.bfloat16
FP32 = mybir.dt.float32
INT16 = mybir.dt.int16

LAST_RESULT = None
_BUILD_CACHE = {}


def _prep(x, edge_index, n_cores, tiles_per_core, table_rows):
    """Host-side graph partitioning + operand packing (all numpy, O(N + E))."""
    n = x.shape[0]
    row = np.asarray(edge_index[0], dtype=np.int64)
    col = np.asarray(edge_index[1], dtype=np.int64)

    deg = np.bincount(col, minlength=n).astype(np.float32) + 1.0  # incl self loop
    dinv = (1.0 / np.sqrt(deg)).astype(np.float32)

    xt = (np.asarray(x, dtype=np.float32) * dinv[:, None]).astype(ml_dtypes.bfloat16)

    n_tiles = n_cores * tiles_per_core
    n_ranges = -(-n // table_rows)

    # ---- assign nodes to (tile, slot): snake order over tiles, nodes by degree desc
    order = np.argsort(-deg, kind="stable")
    fwd = np.arange(n_tiles, dtype=np.int64)
    snake = np.concatenate([fwd, fwd[::-1]])
    reps = (n + 2 * n_tiles - 1) // (2 * n_tiles)
    tile_seq = np.tile(snake, reps)[:n]
    node_tile = np.empty(n, dtype=np.int64)
    node_tile[order] = tile_seq
    t_order = np.argsort(node_tile[order], kind="stable")
    counts = np.bincount(node_tile, minlength=n_tiles)
    assert counts.max() <= 128, f"tile node capacity exceeded: {counts.max()}"
    starts = np.zeros(n_tiles, dtype=np.int64)
    starts[1:] = np.cumsum(counts)[:-1]
    within = np.arange(n, dtype=np.int64) - np.repeat(starts, counts)
    nodes_grouped = order[t_order]
    node_slot = np.empty(n, dtype=np.int64)
    node_slot[nodes_grouped] = within

    # ---- messages: edges + self loops; keyed by (dest tile, src range)
    msrc = np.concatenate([row, np.arange(n, dtype=np.int64)])
    mdst = np.concatenate([col, np.arange(n, dtype=np.int64)])
    m_tile = node_tile[mdst]
    m_slot = node_slot[mdst]
    m_val = dinv[mdst]
    m_rng = msrc // table_rows

    key = m_tile * n_ranges + m_rng
    seg_load = np.bincount(key, minlength=n_tiles * n_ranges)
    cpt = int(-(-seg_load.max() // 128))  # chunks per (tile, range)
    seg_cap = cpt * 128

    ms_order = np.argsort(key, kind="stable")
    seg_starts = np.zeros(n_tiles * n_ranges, dtype=np.int64)
    seg_starts[1:] = np.cumsum(seg_load)[:-1]
    pos = np.arange(len(ms_order), dtype=np.int64) - np.repeat(seg_starts, seg_load)

    # padded [n_tiles * n_ranges, seg_cap]
    src_p = np.zeros((n_tiles * n_ranges, seg_cap), dtype=np.int64)
    dst_p = np.full((n_tiles * n_ranges, seg_cap), 255.0, dtype=np.float32)
    val_p = np.zeros((n_tiles * n_ranges, seg_cap), dtype=np.float32)
    flat = key[ms_order] * seg_cap + pos
    src_p.reshape(-1)[flat] = msrc[ms_order] % table_rows
    dst_p.reshape(-1)[flat] = m_slot[ms_order].astype(np.float32)
    val_p.reshape(-1)[flat] = m_val[ms_order]

    src5 = src_p.reshape(n_cores, tiles_per_core, n_ranges, seg_cap)
    dst5 = dst_p.reshape(n_cores, tiles_per_core, n_ranges, seg_cap)
    val5 = val_p.reshape(n_cores, tiles_per_core, n_ranges, seg_cap)

    idxs, metas = [], []
    stream_len = tiles_per_core * seg_cap  # per range
    for k in range(n_cores):
        # gather stream per range: tiles in order -> [n_ranges, stream_len]
        st = src5[k].transpose(1, 0, 2).reshape(n_ranges, stream_len)
        # idx wrap: position i -> [i % 16, i // 16], replicated across 8 groups of 16
        wrapped = st.reshape(n_ranges, stream_len // 16, 16).transpose(0, 2, 1)
        idx16 = np.tile(wrapped, (1, 8, 1))  # [n_ranges, 128, stream/16]
        idx_all = np.concatenate(list(idx16), axis=1)
        idxs.append(np.ascontiguousarray(idx_all.astype(np.int16)))

        # meta col for (tile t, range r, chunk j) = t*(n_ranges*cpt) + r*cpt + j
        d4 = dst5[k].reshape(tiles_per_core * n_ranges * cpt, 128).T
        v4 = val5[k].reshape(tiles_per_core * n_ranges * cpt, 128).T
        metas.append(np.ascontiguousarray(
            np.concatenate([d4, v4], axis=1).astype(np.float32)))

    return dict(
        xt=xt, dinv=dinv, cpt=cpt, n_ranges=n_ranges,
        idxs=idxs, metas=metas,
        node_tile=node_tile, node_slot=node_slot,
    )


def _build(n_nodes, n_cores, tiles_per_core, ch_per_g, table_rows, n_ranges, cpt):
    """Build the SPMD Bass program (identical across cores)."""
    nc = bacc.Bacc(None, num_devices=n_cores)

    n_chunks_r = tiles_per_core * cpt             # chunks per range stream
    stream_len = n_chunks_r * 128                 # idxs per range stream
    n_g = -(-n_chunks_r // ch_per_g)              # gathers per range
    mcols = tiles_per_core * n_ranges * cpt       # meta columns (per half)

    xt_d = nc.dram_tensor("xt", [n_nodes, D], BF16, kind="ExternalInput")
    idx_d = nc.dram_tensor("idx", [128, n_ranges * stream_len // 16], INT16,
                           kind="ExternalInput")
    meta_d = nc.dram_tensor("meta", [128, 2 * mcols], FP32, kind="ExternalInput")
    wt_d = nc.dram_tensor("wt", [D, D], FP32, kind="ExternalInput")
    gb_d = nc.dram_tensor("gb", [128, 2], FP32, kind="ExternalInput")
    out_d = nc.dram_tensor("out", [128, tiles_per_core * 128], FP32, kind="ExternalOutput")

    cc_in = nc.dram_tensor("cc_in", [128, 2], FP32)
    cc_space = "Shared" if n_cores > 4 else "Local"
    cc_out = nc.dram_tensor("cc_out", [128, 2], FP32, addr_space=cc_space)

    AF = mybir.ActivationFunctionType
    ALU = mybir.AluOpType
    AX = mybir.AxisListType

    with tile.TileContext(nc) as tc:
        with (
            tc.tile_pool(name="const", bufs=1) as cpool,
            tc.tile_pool(name="gbuf", bufs=3) as gpool,
            tc.tile_pool(name="sbuf", bufs=4) as spool,
            tc.tile_pool(name="small", bufs=2) as smpool,
            tc.tile_pool(name="pagg", bufs=2, space="PSUM") as pagg_pool,
            tc.tile_pool(name="pout", bufs=2, space="PSUM") as pout_pool,
        ):
            idx_sb = cpool.tile([128, n_ranges * stream_len // 16], INT16, tag="idx")
            nc.sync.dma_start(out=idx_sb[:], in_=idx_d[:])
            meta_sb = cpool.tile([128, 2 * mcols], FP32, tag="meta")
            nc.sync.dma_start(out=meta_sb[:], in_=meta_d[:])
            iota_i32 = cpool.tile([128, 128], mybir.dt.int32, tag="iota_i32")
            nc.gpsimd.iota(iota_i32[:], pattern=[[1, 128]], base=0, channel_multiplier=0)
            iota_sb = cpool.tile([128, 128], BF16, tag="iota")
            nc.vector.tensor_copy(out=iota_sb[:], in_=iota_i32[:])
            wt_sb = cpool.tile([128, D], FP32, tag="wt")
            nc.sync.dma_start(out=wt_sb[:], in_=wt_d[:])
            gb_sb = cpool.tile([128, 2], FP32, tag="gb")
            nc.sync.dma_start(out=gb_sb[:], in_=gb_d[:])

            pre_bn = cpool.tile([128, tiles_per_core * 128], FP32, tag="prebn")
            sum_sl = cpool.tile([128, tiles_per_core], FP32, tag="sumsl")
            sq_sl = cpool.tile([128, tiles_per_core], FP32, tag="sqsl")

            table_sizes = [min(table_rows, n_nodes - r * table_rows)
                           for r in range(n_ranges)]

            g_tiles = {}

            def get_G(r, g):
                if (r, g) not in g_tiles:
                    nch = min(ch_per_g, n_chunks_r - g * ch_per_g)
                    G = gpool.tile([128, nch * 128], BF16, tag=f"G{r}")
                    a = r * table_rows
                    base = r * stream_len + g * ch_per_g * 128
                    nc.gpsimd.dma_gather(
                        out_ap=G[:].rearrange("p (c f) -> p c f", f=128),
                        in_ap=xt_d[a:a + table_sizes[r], :],
                        idxs_ap=idx_sb[:, base // 16:(base + nch * 128) // 16],
                        num_idxs=nch * 128,
                        num_idxs_reg=nch * 128,
                        elem_size=D,
                    )
                    g_tiles[(r, g)] = G
                return g_tiles[(r, g)]

            for t in range(tiles_per_core):
                pa = pagg_pool.tile([128, 128], FP32, tag="pa")
                nmm = n_ranges * cpt
                for r in range(n_ranges):
                    for j in range(cpt):
                        col = t * nmm + r * cpt + j
                        S = spool.tile([128, 128], BF16, tag="S")
                        # S[e, d] = (iota[d] == dst_slot[e]) * dinv_dst[e]
                        nc.vector.tensor_scalar(
                            out=S[:],
                            in0=iota_sb[:],
                            scalar1=meta_sb[:, col:col + 1],
                            scalar2=meta_sb[:, mcols + col:mcols + col + 1],
                            op0=ALU.is_equal,
                            op1=ALU.mult,
                        )
                        m = r * cpt + j
                        c = t * cpt + j          # chunk within range stream
                        G = get_G(r, c // ch_per_g)
                        gslice = G[:, (c % ch_per_g) * 128:(c % ch_per_g + 1) * 128]
                        nc.tensor.matmul(
                            pa[:], lhsT=gslice, rhs=S[:],
                            start=(m == 0), stop=(m == nmm - 1),
                        )

                agg = spool.tile([128, 128], FP32, tag="agg")
                nc.vector.tensor_copy(out=agg[:], in_=pa[:])
                po = pout_pool.tile([128, 128], FP32, tag="po")
                nc.tensor.matmul(po[:], lhsT=wt_sb[:], rhs=agg[:], start=True, stop=True)

                nc.vector.tensor_reduce(
                    out=sum_sl[:, t:t + 1], in_=po[:], axis=AX.X, op=ALU.add
                )
                sq = spool.tile([128, 128], FP32, tag="sq")
                nc.scalar.activation(
                    out=sq[:], in_=po[:], func=AF.Square,
                    accum_out=sq_sl[:, t:t + 1],
                )
                nc.vector.tensor_copy(out=pre_bn[:, t * 128:(t + 1) * 128], in_=po[:])

            # ---- BN stats: local reduce, all-reduce, scale/shift
            stats = smpool.tile([128, 2], FP32, tag="stats")
            nc.vector.tensor_reduce(out=stats[:, 0:1], in_=sum_sl[:], axis=AX.X, op=ALU.add)
            nc.vector.tensor_reduce(out=stats[:, 1:2], in_=sq_sl[:], axis=AX.X, op=ALU.add)
            nc.sync.dma_start(out=cc_in[:], in_=stats[:])
            nc.gpsimd.collective_compute(
                "AllReduce", ALU.add,
                replica_groups=[list(range(n_cores))],
                ins=[cc_in[:]], outs=[cc_out[:]],
            )
            statg = smpool.tile([128, 2], FP32, tag="statg")
            nc.sync.dma_start(out=statg[:], in_=cc_out[:])

            mean = smpool.tile([128, 1], FP32, tag="mean")
            nc.vector.tensor_scalar_mul(mean[:], statg[:, 0:1], 1.0 / n_nodes)
            ex2 = smpool.tile([128, 1], FP32, tag="ex2")
            nc.vector.tensor_scalar_mul(ex2[:], statg[:, 1:2], 1.0 / n_nodes)
            m2 = smpool.tile([128, 1], FP32, tag="m2")
            nc.vector.tensor_tensor(out=m2[:], in0=mean[:], in1=mean[:], op=ALU.mult)
            var = smpool.tile([128, 1], FP32, tag="var")
            nc.vector.tensor_tensor(out=var[:], in0=ex2[:], in1=m2[:], op=ALU.subtract)
            nc.vector.tensor_scalar_add(var[:], var[:], BN_EPS)
            inv = smpool.tile([128, 1], FP32, tag="inv")
            nc.vector.reciprocal(inv[:], var[:])
            istd = smpool.tile([128, 1], FP32, tag="istd")
            nc.scalar.sqrt(istd[:], inv[:])
            scale = smpool.tile([128, 1], FP32, tag="scale")
            nc.vector.tensor_tensor(out=scale[:], in0=gb_sb[:, 0:1], in1=istd[:], op=ALU.mult)
            msc = smpool.tile([128, 1], FP32, tag="msc")
            nc.vector.tensor_tensor(out=msc[:], in0=mean[:], in1=scale[:], op=ALU.mult)
            shift = smpool.tile([128, 1], FP32, tag="shift")
            nc.vector.tensor_tensor(out=shift[:], in0=gb_sb[:, 1:2], in1=msc[:], op=ALU.subtract)

            for t in range(tiles_per_core):
                nc.scalar.activation(
                    out=pre_bn[:, t * 128:(t + 1) * 128],
                    in_=pre_bn[:, t * 128:(t + 1) * 128],
                    func=AF.Relu, scale=scale[:], bias=shift[:],
                )
            nc.sync.dma_start(out=out_d[:], in_=pre_bn[:])

    nc.compile()
    return nc


def _get_program(n_nodes, n_cores, tiles_per_core, ch_per_g, table_rows, n_ranges, cpt):
    key = (n_nodes, n_cores, tiles_per_core, ch_per_g, table_rows, n_ranges, cpt)
    if key not in _BUILD_CACHE:
        _BUILD_CACHE[key] = _build(*key)
    return _BUILD_CACHE[key]


def kernel(x, edge_index, W, b, gamma, beta, _run_fn=None):
    x = np.asarray(x, dtype=np.float32)
    edge_index = np.asarray(edge_index)
    W = np.asarray(W, dtype=np.float32)
    gamma = np.asarray(gamma, dtype=np.float32)
    beta = np.asarray(beta, dtype=np.float32)

    n = x.shape[0]
    assert n == N_NODES and x.shape[1] == D

    plan = _prep(x, edge_index, N_CORES, TILES_PER_CORE, TABLE_ROWS)

    wt = np.ascontiguousarray(W.T.astype(np.float32))  # [in_f, out_o]
    gb = np.stack([gamma, beta], axis=1).astype(np.float32)

    in_maps = []
    for k in range(N_CORES):
        in_maps.append(dict(
            xt=plan["xt"], idx=plan["idxs"][k], meta=plan["metas"][k],
            wt=wt, gb=gb,
        ))

    nc = _get_program(n, N_CORES, TILES_PER_CORE, CH_PER_G, TABLE_ROWS,
                      plan["n_ranges"], plan["cpt"])

    global LAST_RESULT
    if _run_fn is not None:
        results = _run_fn(nc, in_maps)
    else:
        LAST_RESULT = run_bass_kernel_spmd(nc, in_maps, core_ids=list(range(N_CORES)))
        results = LAST_RESULT.results

    # ---- unshard: out[k] is [128 feat, tiles*128 slots]
    node_tile = plan["node_tile"]
    node_slot = plan["node_slot"]
    y = np.empty((n, D), dtype=np.float32)
    for k in range(N_CORES):
        sel = np.where((node_tile // TILES_PER_CORE) == k)[0]
        cols = (node_tile[sel] % TILES_PER_CORE) * 128 + node_slot[sel]
        yk = np.asarray(results[k]["out"], dtype=np.float32)
        y[sel] = yk[:, cols].T
    return y
